# revision 22
# baseline (speedup 1.0000x reference)
"""AdaptiveTopKChannelStack (SG-MoE noisy-gate monotonic top-k) on 8 TRN2 NeuronCores.

Data-parallel over batch: each core handles 4096 of the 32768 rows.
Per core:
  - gate:  H = x @ Wg + noise_eps * softplus(x @ Wn). The gate matmul runs
           as a 3-term bf16 hi/lo split (x = xh + xl, W = Wh + Wl host-side;
           xh@Wh + xh@Wl + xl@Wh accumulate in fp32 PSUM; the dropped
           xl@Wl term is O(2^-18)), giving |H err| ~4e-5, below the
           minimum top-2 gap of H, so the argmax matches the fp32
           reference exactly while keeping the whole PE stream bf16
           (mixing fp32 and bf16 matmuls trips a hardware FWL/FP32
           hazard that faults the exec unit). The two xh terms run as ONE
           64-wide matmul (into disjoint psum columns, summed later on
           DVE) so the gate costs 2 PE instructions per chunk, riding
           inside the 512-wide main weight streams.
           softplus = ln(1 + exp(z)) directly (z = x@Wn is ~N(0,1);
           |z| < 6 so exp never overflows): Exp + Ln(bias=1) on the
           scalar engine only.
  - route: k = argmax(H) via prefix-max scan; threshold t = 128*(k+1)
  - main:  y = x @ Wc_flat  (bf16 = xh @ Wc, fp32 accumulate in PSUM)
  - epilogue, no psum->sbuf copy at all:
           mask16 = (expert_iota <= k)  gpsimd tensor_scalar (DVE on the
                                        last tile for tail latency)
           out    = mask16 * psum_f32   DVE tensor_tensor, reads PSUM
                                        directly, downcasts fp16 on write
    fp16 output halves the output DMA vs fp32.
Startup: input DMA issues are spread across 4 engine queues (sync: xh,
scalar: wc, vector: xl, gpsimd: whl/eps/iota) so the ~0.7us-per-issue
serialization doesn't gate the first tile; ~8 dummy matmuls on a
memset tile run during the initial DMA wait to trip the PE HAM
activity window so real tiles run at 2.4 GHz instead of the cold
1.2 GHz default.
Tail: per-tile epilogue is emitted gate-first (the 3 tiny gate ops
don't queue behind the 1.1us psum copies on the scalar FIFO), the last
chunk runs its gate matmuls before its mains (threshold ready ~0.9us
earlier), and the last tile splits mult+DMA by halves.
x is transposed host-side so the contraction dim lands on SBUF partitions
with fully contiguous DMA; weights are replicated to all cores.
"""

import numpy as np
import ml_dtypes

import concourse.bass as bass
import concourse.mybir as mybir
import concourse.tile as tile
from concourse.bass_utils import run_bass_kernel_spmd

F32 = mybir.dt.float32
BF16 = mybir.dt.bfloat16
F16 = mybir.dt.float16
I32 = mybir.dt.int32
AF = mybir.ActivationFunctionType
OP = mybir.AluOpType

N_CORES = 8
B, D, E, CH = 32768, 512, 16, 128
NF = E * CH              # 2048 out features
BLOC = B // N_CORES      # 4096 rows per core
KC = D // 128            # 4 contraction chunks
NT = BLOC // 128         # 32 row tiles per core
N_WARM = 14              # dummy matmuls to warm the PE clock


def _split_multi_waits(nc, max_waits=1):
    """walrus rejects instructions with more than a couple of semaphore
    waits; hoist extra waits into single-wait NOPs ahead of the instruction
    (same engine executes in order, so semantics are unchanged)."""
    for f in nc.m.functions:
        for bb in f.blocks:
            new_insts = []
            for inst in bb.instructions:
                si = inst.sync_info
                if si is not None and si.on_wait and len(si.on_wait) > max_waits:
                    waits = list(si.on_wait)
                    for j, w in enumerate(waits[max_waits:]):
                        new_insts.append(mybir.InstNoOp(
                            name=f"{inst.name}-waitsplit-{j}",
                            sync_info=mybir.SyncInfo(on_wait=[w], on_update=[]),
                            bass_nofuse=True,
                            engine=inst.engine,
                        ))
                    si.on_wait = waits[:max_waits]
                new_insts.append(inst)
            bb.instructions[:] = new_insts


def _build(has_gate_bias, has_comp_bias, split_waits=True):
    nc = bass.Bass("TRN2", target_bir_lowering=False, debug=False)

    xh_ext = nc.declare_dram_parameter("xh", [D, BLOC], BF16, isOutput=False)
    xl_ext = nc.declare_dram_parameter("xl", [D, BLOC], BF16, isOutput=False)
    wc_ext = nc.declare_dram_parameter("wc", [D, NF], BF16, isOutput=False)
    # [Wh | Wl] hi/lo split of [Wg|Wn], concatenated along the free dim
    whl_ext = nc.declare_dram_parameter("whl", [D, 4 * E], BF16, isOutput=False)
    eps_ext = nc.declare_dram_parameter("eps", [1, E], F32, isOutput=False)
    if has_gate_bias:
        gb_ext = nc.declare_dram_parameter("gb", [1, 2 * E], F32, isOutput=False)
    if has_comp_bias:
        bc_ext = nc.declare_dram_parameter("bc", [1, NF], F32, isOutput=False)
    out_ext = nc.declare_dram_parameter("out", [BLOC, NF], F16, isOutput=True)

    with tile.TileContext(nc) as tc:
        with (
            tc.tile_pool(name="big", bufs=1) as big,
            tc.tile_pool(name="outp", bufs=5) as outp,
            # 12 untagged slots: the ~10 tiny gate tiles per iteration
            # otherwise share 3 slots and serialize across iterations on
            # WAR hazards (slots are 64B/partition, so this is free)
            tc.tile_pool(name="small", bufs=12) as small,
            tc.tile_pool(name="ps", bufs=3, space="PSUM") as ps,
            tc.tile_pool(name="psg", bufs=2, space="PSUM") as psg,
        ):
            # ---- resident tensors ----
            xh_t = big.tile([128, KC, BLOC], BF16)
            xl_t = big.tile([128, KC, BLOC], BF16)
            wc_t = big.tile([128, KC, NF], BF16)
            whl_t = big.tile([128, KC, 4 * E], BF16)
            eps_t = big.tile([128, E], F32)
            iota_i = big.tile([128, NF], I32)
            # fp16 expert-index iota (0..15, each repeated CH times):
            # mask[b, e*CH+c] = (e <= k_b) needs only the expert id, which
            # folds the 128*(k+1) threshold math into the compare
            iotae_h = big.tile([128, NF], F16)
            warm_t = big.tile([128, 512], BF16)

            # PE warmup: memset + dummy matmuls keep the PE busy through
            # the HAM activity window during the initial DMA wait, so the
            # first real tiles run at 2.4 GHz.
            nc.gpsimd.memset(warm_t, 0.0)
            warm_ps = ps.tile([128, 512], F32, tag="ps_half", name="ps_warm")
            for w in range(N_WARM):
                nc.tensor.matmul(warm_ps, warm_t[:, 0:128], warm_t,
                                 start=True, stop=True)

            # Input DMA issues spread across engine queues (~0.7us per
            # issue, serialized per queue): sync carries xh, vector xl,
            # scalar wc, gpsimd the small stuff. Ordered so tile 0's
            # working set (xh/xl rows 0:128, whl, wc chunk 0) lands first.
            xh_r = xh_ext.ap().rearrange("(c p) e -> p c e", p=128)
            xl_r = xl_ext.ap().rearrange("(c p) e -> p c e", p=128)
            wc_r = wc_ext.ap().rearrange("(c p) e -> p c e", p=128)

            # gate weights + eps on gpsimd (tiny; the software DMA path
            # is too slow for anything bulky)
            nc.gpsimd.dma_start(out=whl_t, in_=whl_ext.ap().rearrange(
                "(c p) e -> p c e", p=128))
            eps_bc = bass.AP(tensor=eps_ext, offset=0, ap=[[0, 128], [1, E]])
            nc.gpsimd.dma_start(out=eps_t, in_=eps_bc)

            # bulk inputs split across the two hardware DMA queue groups:
            # wc (2.1MB, needed in full by tile 0's 4th chunk) rides the
            # sync group, the x blocks ride the scalar group. wc chunk 0
            # is split in halves so tile 0's first mains start on a 256KB
            # transfer. Blocks below 256 cols are counterproductive (the
            # DMA line is cols*2 bytes; tiny lines collapse efficiency).
            nc.scalar.dma_start(out=wc_t[:, 0, 0:1024], in_=wc_r[:, 0, 0:1024])
            nc.sync.dma_start(out=wc_t[:, 0, 1024:], in_=wc_r[:, 0, 1024:])
            nc.sync.dma_start(out=wc_t[:, 1, :], in_=wc_r[:, 1, :])
            nc.sync.dma_start(out=wc_t[:, 2, :], in_=wc_r[:, 2, :])
            nc.sync.dma_start(out=wc_t[:, 3, :], in_=wc_r[:, 3, :])

            xblocks = [256, 256, 512, 1024, 1024, 1024]
            start = 0
            for j, blk in enumerate(xblocks):
                cols = slice(start, start + blk)
                start += blk
                # first two block pairs (tiles 0-3) on the scalar queue so
                # they overlap the wc stream; the rest on sync (the scalar
                # queue must be free for per-tile epilogue work by ~14us)
                q = nc.scalar if j < 2 else nc.sync
                q.dma_start(out=xh_t[:, :, cols], in_=xh_r[:, :, cols])
                q.dma_start(out=xl_t[:, :, cols], in_=xl_r[:, :, cols])

            if has_gate_bias:
                gb_t = big.tile([128, 2 * E], F32)
                nc.gpsimd.dma_start(out=gb_t, in_=bass.AP(
                    tensor=gb_ext, offset=0, ap=[[0, 128], [1, 2 * E]]))
            if has_comp_bias:
                bc_t = big.tile([128, NF], F32)
                nc.gpsimd.dma_start(out=bc_t, in_=bass.AP(
                    tensor=bc_ext, offset=0, ap=[[0, 128], [1, NF]]))
            nc.gpsimd.iota(iota_i, pattern=[[1, E], [0, CH]], base=0,
                           channel_multiplier=0)
            nc.gpsimd.tensor_copy(iotae_h, iota_i)

            # ---- per-row-tile pipeline ----
            # The mask/mult/DMA stage is software-pipelined one tile late:
            # tile i-1's big DVE ops are emitted during tile i, so the
            # exposed tail after the last matmul is just one mask+mult+DMA
            # instead of two full epilogues serialized on the DVE FIFO.
            pending = []

            def flush_pending(last):
                (pi, py16, pmask) = pending.pop(0)
                prows = slice(pi * 128, (pi + 1) * 128)
                o_t = outp.tile([128, NF], F16, tag="o", name=f"o_{pi}")
                if not last:
                    nc.vector.tensor_tensor(out=o_t, in0=py16, in1=pmask,
                                            op=OP.mult)
                    nc.sync.dma_start(out=out_ext[prows, :], in_=o_t)
                else:
                    # last tile: per-half mult+DMA (h1 via the scalar hw
                    # queue) so issues and transfers overlap
                    for h in range(2):
                        hc = slice(h * 1024, (h + 1) * 1024)
                        nc.vector.tensor_tensor(out=o_t[:, hc],
                                                in0=py16[:, hc],
                                                in1=pmask[:, hc], op=OP.mult)
                        q = nc.sync if h == 0 else nc.scalar
                        q.dma_start(out=out_ext[prows, hc], in_=o_t[:, hc])

            for i in range(NT):
                rows = slice(i * 128, (i + 1) * 128)

                # gate accumulates 3 bf16 hi/lo terms as 2 matmuls per
                # chunk: xh@[Wh|Wl] (64-wide) into ps_g[:, 0:64] and
                # xl@Wh (32-wide) into ps_g[:, 64:96]; the three 32-col
                # slices are summed on DVE in the epilogue. The matmuls
                # ride inside the main weight stream so their LDWEIGHTS
                # hide under the 512-wide main streams via the PE's
                # reorder window.
                ps_g = psg.tile([128, 6 * E], F32)
                ps_h = [ps.tile([128, 1024], F32, tag="ps_half",
                                name=f"ps_half_{i}_{h}") for h in range(2)]

                def gate_mms(c):
                    # start=True clears has_written for the WHOLE psum
                    # bank, so only the very first matmul of the bank may
                    # carry it; the xl term's first write lands on
                    # freshly-cleared columns and overwrites via the
                    # per-element has_written bits.
                    nc.tensor.matmul(
                        ps_g[:, 0:4 * E], xh_t[:, c, rows],
                        whl_t[:, c, 0:4 * E],
                        start=(c == 0), stop=(c == KC - 1))
                    nc.tensor.matmul(
                        ps_g[:, 4 * E:6 * E], xl_t[:, c, rows],
                        whl_t[:, c, 0:2 * E],
                        start=False, stop=(c == KC - 1),
                        skip_group_check=(c == 0))

                def main_mms(c):
                    for n in range(4):
                        nc.tensor.matmul(
                            ps_h[n // 2][:, (n % 2) * 512:(n % 2) * 512 + 512],
                            xh_t[:, c, rows],
                            wc_t[:, c, n * 512:(n + 1) * 512],
                            start=(c == 0), stop=(c == KC - 1))

                # last chunk runs the gate first so the threshold chain
                # starts ~0.9us before the tile's matmuls finish
                for c in range(KC):
                    if c == KC - 1:
                        gate_mms(c)
                        main_mms(c)
                    else:
                        main_mms(c)
                        gate_mms(c)

                # ---- gate epilogue first (tiny ops must not queue
                # behind the 1.1us psum copies on the engine FIFOs) ----
                # sum the 3 hi/lo product slices: gn = [g|n] fp32. DVE
                # tensor_tensor may read only ONE operand from PSUM, so
                # the middle slice goes through a scalar-engine copy.
                c1_t = small.tile([128, 2 * E], F32)
                nc.scalar.activation(c1_t, ps_g[:, 2 * E:4 * E], AF.Copy)
                s1_t = small.tile([128, 2 * E], F32)
                nc.vector.tensor_tensor(out=s1_t, in0=ps_g[:, 0:2 * E],
                                        in1=c1_t, op=OP.add)
                gn_t = small.tile([128, 2 * E], F32)
                nc.vector.tensor_tensor(out=gn_t, in0=ps_g[:, 4 * E:6 * E],
                                        in1=s1_t, op=OP.add)
                if has_gate_bias:
                    gnb_t = small.tile([128, 2 * E], F32)
                    nc.vector.tensor_tensor(out=gnb_t, in0=gn_t, in1=gb_t,
                                            op=OP.add)
                    gn_t = gnb_t
                g_ps, n_ps = gn_t[:, 0:E], gn_t[:, E:2 * E]
                # softplus(z) = ln(1 + exp(z)); z ~ N(0,1) so exp(z) is
                # far from fp32 overflow. Two scalar-engine ops.
                ex_t = small.tile([128, E], F32)
                nc.scalar.activation(ex_t, n_ps, AF.Exp)
                sp_t = small.tile([128, E], F32)
                nc.scalar.activation(sp_t, ex_t, AF.Ln, bias=1.0)
                he_t = small.tile([128, E], F32)
                nc.gpsimd.tensor_tensor(out=he_t, in0=sp_t, in1=eps_t, op=OP.mult)
                # the tiny h/bits ops go to gpsimd mid-kernel to unload the
                # DVE; on the last tile they stay on DVE (shorter
                # cross-engine chain on the exposed tail)
                seng = nc.vector if i == NT - 1 else nc.gpsimd
                h_t = small.tile([128, E], F32)
                seng.tensor_tensor(out=h_t, in0=he_t, in1=g_ps, op=OP.add)
                pm_t = small.tile([128, E], F32)
                nc.vector.tensor_tensor_scan(pm_t, h_t, h_t, initial=-1e30,
                                             op0=OP.max, op1=OP.bypass)
                bits_t = small.tile([128, E], F32)
                ks_t = small.tile([128, 1], F32)
                # (tensor_scalar with an AP scalar is DVE-only)
                nc.vector.tensor_scalar(out=bits_t, in0=pm_t,
                                        scalar1=pm_t[:, E - 1:E], scalar2=0.0,
                                        op0=OP.is_lt, op1=OP.add, accum_out=ks_t)
                # masked epilogue: no psum->sbuf copy; the DVE mult reads
                # PSUM fp32 directly and downcasts to fp16 on write.
                #   mask16 = (expert_iota <= k)   gpsimd (DVE for last tile)
                #   out    = mask16 * psum        DVE tensor_tensor
                mask_t = small.tile([128, NF], F16, tag="mask", bufs=3,
                                    name=f"mask_{i}")
                nc.vector.tensor_scalar(out=mask_t, in0=iotae_h,
                                        scalar1=ks_t[:, 0:1], scalar2=None,
                                        op0=OP.is_le)

                # psum -> fp16 via scalar-engine ACT copies (prompt psum
                # release keeps the main matmul pipeline fed), then DVE
                # mask-mult in fp16 2x mode
                y16_t = small.tile([128, NF], F16, tag="y16", bufs=3,
                                   name=f"y16_{i}")

                def copy_half(h):
                    src = ps_h[h]
                    if has_comp_bias:
                        src = small.tile([128, 1024], F32, tag="biased", bufs=3,
                                         name=f"biased_{i}_{h}")
                        nc.vector.tensor_tensor(
                            out=src, in0=ps_h[h],
                            in1=bc_t[:, h * 1024:(h + 1) * 1024], op=OP.add)
                    nc.scalar.activation(y16_t[:, h * 1024:(h + 1) * 1024],
                                         src, AF.Copy)

                copy_half(0)
                copy_half(1)

                pending.append((i, y16_t, mask_t))
                if len(pending) > 1:
                    flush_pending(last=False)
            flush_pending(last=True)

    if split_waits:
        _split_multi_waits(nc)
    return nc


_NC_CACHE = {}


def kernel(x, Wc, bc, Wg_w, Wg_b, Wn_w, Wn_b, noise_eps):
    x = np.ascontiguousarray(np.asarray(x, dtype=np.float32))
    Wc = np.asarray(Wc, dtype=np.float32)
    bc = np.asarray(bc, dtype=np.float32)
    Wg_w = np.asarray(Wg_w, dtype=np.float32)
    Wg_b = np.asarray(Wg_b, dtype=np.float32)
    Wn_w = np.asarray(Wn_w, dtype=np.float32)
    Wn_b = np.asarray(Wn_b, dtype=np.float32)
    noise_eps = np.asarray(noise_eps, dtype=np.float32)

    has_gate_bias = bool(np.any(Wg_b) or np.any(Wn_b))
    has_comp_bias = bool(np.any(bc))

    key = (has_gate_bias, has_comp_bias)
    if key not in _NC_CACHE:
        _NC_CACHE[key] = _build(has_gate_bias, has_comp_bias)
    nc = _NC_CACHE[key]

    bf = ml_dtypes.bfloat16
    xT = x.T                                   # [D, B]
    xh = xT.astype(bf)
    xl = (xT - xh.astype(np.float32)).astype(bf)
    wgn = np.concatenate([Wg_w, Wn_w], axis=1)  # [D, 2E] fp32
    wgh = wgn.astype(bf)
    wgl = (wgn - wgh.astype(np.float32)).astype(bf)
    whl = np.ascontiguousarray(np.concatenate([wgh, wgl], axis=1))  # [D, 4E]
    wc_flat = np.ascontiguousarray(
        Wc.transpose(1, 0, 2).reshape(D, NF).astype(bf))
    eps2 = np.ascontiguousarray(noise_eps.reshape(1, E))

    in_maps = []
    for i in range(N_CORES):
        cols = slice(i * BLOC, (i + 1) * BLOC)
        m = {
            "xh": np.ascontiguousarray(xh[:, cols]),
            "xl": np.ascontiguousarray(xl[:, cols]),
            "wc": wc_flat,
            "whl": whl,
            "eps": eps2,
        }
        if has_gate_bias:
            m["gb"] = np.ascontiguousarray(
                np.concatenate([Wg_b, Wn_b]).reshape(1, 2 * E).astype(np.float32))
        if has_comp_bias:
            m["bc"] = np.ascontiguousarray(bc.reshape(1, NF).astype(np.float32))
        in_maps.append(m)

    res = run_bass_kernel_spmd(nc, in_maps, core_ids=list(range(N_CORES)))
    out = np.concatenate(
        [np.asarray(res.results[i]["out"]).astype(np.float32)
         for i in range(N_CORES)], axis=0)
    return out
